# revision 11
# baseline (speedup 1.0000x reference)
"""TRN2 Bass kernel for nn_GTLayer (ELL sparse attention, N=50000, K=16).

Sharding: nodes split contiguously across 8 NeuronCores (6250/core, padded
to 6272 = 49 x 128), with each core's nodes re-ordered by unmasked-neighbor
count (outputs un-permuted on the host). Per core, three phases:
  1. q|k|v via host-fused lookup tables: T_f[v] = emb_f[v] @ [Wq|Wk|Wv]
     (+biases), feature-PAIR-packed -> 5 indirect-DMA row fetches per
     128-node tile + an f16 add-tree. No matmuls on device.
  2. AllGather of kv rows across the 8 cores (index/mask loads hidden).
  3. per tile: only the J_t = max-unmasked-count neighbor rows are gathered
     (masked neighbors contribute exp(-9)~0 and are skipped entirely;
     short rows are padded with dummy slots masked to 0). Attention runs
     in f16 on DVE with binary add-trees; exp on the Act engine.
Masking: tt=(s+36)*mask, e=exp(0.25*tt-9); dummy lanes get exp(-9)~1.2e-4.
Fully-masked rows are patched exactly on the host (mean of v over the
original neighbor list).
"""
import numpy as np

import concourse.bass as bass
import concourse.mybir as mybir
import concourse.tile as tile
from concourse.vector_clock import ScopedClock

F32 = mybir.dt.float32
I32 = mybir.dt.int32
F16 = mybir.dt.float16
AX = mybir.AxisListType
ALU = mybir.AluOpType
AF = mybir.ActivationFunctionType

N_FEATS, VOCAB, HID, NH, HD, K = 9, 119, 128, 8, 16, 16
P = 128
NCORES = 8
NRC = 6250          # real nodes per core
NPC = 6272          # padded nodes per core (49 x 128)
T = NPC // P
NPAIR = VOCAB * VOCAB
# fused q|k|v lookup table: 4 pair tables + 1 single table, 384-wide f16 rows
EMB_ROWS = 4 * NPAIR + VOCAB
QKV = 3 * HID

# ---------------------------------------------------------------- walrus fixes
# This walrus build rejects >1 sync-wait command per instruction. Two fixes:
# (1) TileContext tail drain: emit waits as single-wait nops.
# (2) General: split multi-wait instructions in the serialized BIR JSON by
#     inserting single-wait NoOps immediately before them (order preserved).


def _patched_drain_and_barrier(self, tick_clock, wait_clock):
    nc = self.nc
    probe = nc.sync.nop(nofuse=True)
    wait_clock.add_sem_waits(probe.ins, ScopedClock({None: tick_clock.global_clock}))
    waits = list(probe.ins.sync_info.on_wait or []) if probe.ins.sync_info else []
    if probe.ins.sync_info:
        probe.ins.sync_info.on_wait = waits[:1]
    for w in waits[1:]:
        n2 = nc.sync.nop(nofuse=True)
        if n2.ins.sync_info is None:
            n2.ins.sync_info = mybir.SyncInfo(on_update=[], on_wait=[w])
        else:
            n2.ins.sync_info.on_wait = [w]
    nc.sync.drain()
    nc.all_engine_barrier()
    assert self.sems is not None
    popped = nc._tile_sem_poison_stack.pop()
    assert popped is self._sem_poison
    nc.clear_and_free_semaphores(list(self.sems.allocated().values()))
    nc.all_engine_barrier()


tile.TileContext._drain_and_barrier = _patched_drain_and_barrier


def _split_waits_json(bir_bytes):
    import orjson
    m = orjson.loads(bir_bytes)
    n = 0
    for fn in m["functions"]:
        for blk in fn["blocks"]:
            new = []
            for ins in blk["instructions"]:
                si = ins.get("sync_info")
                waits = (si or {}).get("on_wait") or []
                if len(waits) > 1:
                    for w in waits[:-1]:
                        n += 1
                        new.append({
                            "debug": ins.get("debug", 0),
                            "engine": ins["engine"],
                            "ins": [], "name": f"I-wfix-{n}",
                            "opcode": "NoOp", "outs": [],
                            "sync_info": {"on_update": [], "on_wait": [w]},
                        })
                    si["on_wait"] = waits[-1:]
                new.append(ins)
            blk["instructions"] = new
    return orjson.dumps(m), n


import concourse.bass2jax as _b2j

_orig_cbk = _b2j.compile_bir_kernel


def _patched_cbk(ant_bir_str, *a, **kw):
    fixed, n = _split_waits_json(ant_bir_str)
    return _orig_cbk(fixed, *a, **kw)


_b2j.compile_bir_kernel = _patched_cbk

# ---------------------------------------------------------------- device code


def build(nc, j_list, sumj):
    ntot = NPC * NCORES

    xc = nc.dram_tensor("xc", [P, T * 5], I32, kind="ExternalInput")
    nb = nc.dram_tensor("nb", [P, sumj], I32, kind="ExternalInput")
    mk = nc.dram_tensor("mk", [P, sumj], F16, kind="ExternalInput")
    emb = nc.dram_tensor("emb", [EMB_ROWS, QKV], F16, kind="ExternalInput")
    out = nc.dram_tensor("out", [NPC, HID], F32, kind="ExternalOutput")

    joff = np.concatenate([[0], np.cumsum(j_list)]).astype(int)

    lp = nc.allow_low_precision(reason="f16 attention pipeline")
    lp.__enter__()
    with tile.TileContext(nc) as tc:
        with (
            tc.tile_pool(name="const", bufs=1) as cp,
            tc.tile_pool(name="resident", bufs=1) as rp,
            tc.tile_pool(name="work", bufs=3) as wp,
            tc.tile_pool(name="gath", bufs=3) as gp,
            tc.tile_pool(name="dram", bufs=1, space="DRAM") as dp,
        ):
            negq = cp.tile([P, 1], F32, name="negq")
            nc.gpsimd.memset(negq[:], -9.0)

            xt_all = rp.tile([P, T * 5], I32, name="xt_all")
            nc.sync.dma_start(out=xt_all[:], in_=xc[:])
            idx_all = rp.tile([P, sumj], I32, name="idx_all")
            msk_all = rp.tile([P, sumj], F16, name="msk_all")
            qkv_all = rp.tile([P, T * QKV], F16, name="qkv_all")

            kv_shard = dp.tile([NPC, 2 * HID], F16, name="kv_shard")
            kv_full = dp.tile([ntot, 2 * HID], F16, name="kv_full",
                              addr_space="Shared")

            # ---------------- phase 1: fused q|k|v lookup + add-tree
            for t in range(T):
                r0 = t * P
                et = gp.tile([P, 5 * QKV], F16, name="et")
                for c in range(5):
                    nc.gpsimd.indirect_dma_start(
                        out=et[:, c * QKV:(c + 1) * QKV], out_offset=None,
                        in_=emb[:],
                        in_offset=bass.IndirectOffsetOnAxis(
                            ap=xt_all[:, t * 5 + c:t * 5 + c + 1], axis=0))
                eL1 = wp.tile([P, 2 * QKV], F16, name="eL1")
                nc.vector.tensor_tensor(
                    out=eL1[:], in0=et[:, 0:2 * QKV],
                    in1=et[:, 2 * QKV:4 * QKV], op=ALU.add)
                eL2 = wp.tile([P, QKV], F16, name="eL2")
                nc.vector.tensor_tensor(
                    out=eL2[:], in0=eL1[:, 0:QKV], in1=eL1[:, QKV:2 * QKV],
                    op=ALU.add)
                nc.vector.tensor_tensor(
                    out=qkv_all[:, t * QKV:(t + 1) * QKV], in0=eL2[:],
                    in1=et[:, 4 * QKV:5 * QKV], op=ALU.add)
            nc.sync.dma_start(
                out=kv_shard[:].rearrange("(t p) c -> p t c", p=P),
                in_=qkv_all[:].rearrange(
                    "p (t c) -> p t c", t=T)[:, :, HID:QKV])

            # ---------------- phase 2: allgather kv; loads hidden under it
            nc.gpsimd.collective_compute(
                "AllGather", ALU.bypass,
                replica_groups=[list(range(NCORES))],
                ins=[kv_shard[:]], outs=[kv_full[:]])

            nc.sync.dma_start(out=idx_all[:], in_=nb[:])
            nc.sync.dma_start(out=msk_all[:], in_=mk[:])

            # ---------------- phase 3: J_t neighbor gathers + attention
            def gather(t):
                J = j_list[t]
                knvn = gp.tile([P, K * 2 * HID], F16, name="knvn")
                for j in range(J):
                    o = joff[t] + j
                    nc.gpsimd.indirect_dma_start(
                        out=knvn[:, j * 2 * HID:(j + 1) * 2 * HID],
                        out_offset=None, in_=kv_full[:],
                        in_offset=bass.IndirectOffsetOnAxis(
                            ap=idx_all[:, o:o + 1], axis=0))
                return knvn

            knvn_next = gather(0)
            for t in range(T):
                r0 = t * P
                J = j_list[t]
                knvn = knvn_next
                if t + 1 < T:
                    knvn_next = gather(t + 1)
                kview = knvn[:].rearrange("p (j c) -> p j c", j=K)
                kn = kview[:, 0:J, 0:HID]
                vn = kview[:, 0:J, HID:2 * HID]

                qb = qkv_all[:, t * QKV:t * QKV + HID] \
                    .rearrange("p (a c) -> p a c", a=1).to_broadcast([P, J, HID])
                prod = wp.tile([P, K * HID], F16, name="prod")
                nc.vector.tensor_tensor(
                    out=prod[:, 0:J * HID].rearrange("p (j c) -> p j c", j=J),
                    in0=kn, in1=qb, op=ALU.mult)

                # score tree over head_dim d: 16 -> 8 -> 4 -> 2 -> 1
                pv = prod[:, 0:J * HID].rearrange(
                    "p (j h d) -> p j h d", j=J, h=NH)
                sL1 = wp.tile([P, K * NH * 8], F16, name="sL1")
                nc.vector.tensor_tensor(
                    out=sL1[:, 0:J * NH * 8].rearrange(
                        "p (j h d) -> p j h d", j=J, h=NH),
                    in0=pv[:, :, :, 0:8], in1=pv[:, :, :, 8:16], op=ALU.add)
                s1v = sL1[:, 0:J * NH * 8].rearrange(
                    "p (j h d) -> p j h d", j=J, h=NH)
                sL2 = wp.tile([P, K * NH * 4], F16, name="sL2")
                nc.vector.tensor_tensor(
                    out=sL2[:, 0:J * NH * 4].rearrange(
                        "p (j h d) -> p j h d", j=J, h=NH),
                    in0=s1v[:, :, :, 0:4], in1=s1v[:, :, :, 4:8], op=ALU.add)
                s2v = sL2[:, 0:J * NH * 4].rearrange(
                    "p (j h d) -> p j h d", j=J, h=NH)
                sL3 = wp.tile([P, K * NH * 2], F16, name="sL3")
                nc.vector.tensor_tensor(
                    out=sL3[:, 0:J * NH * 2].rearrange(
                        "p (j h d) -> p j h d", j=J, h=NH),
                    in0=s2v[:, :, :, 0:2], in1=s2v[:, :, :, 2:4], op=ALU.add)
                s3v = sL3[:, 0:J * NH * 2].rearrange(
                    "p (j h d) -> p j h d", j=J, h=NH)
                s = wp.tile([P, K * NH], F32, name="s")
                nc.vector.tensor_tensor(
                    out=s[:, 0:J * NH].rearrange(
                        "p (j h a) -> p j h a", j=J, h=NH),
                    in0=s3v[:, :, :, 0:1], in1=s3v[:, :, :, 1:2], op=ALU.add)

                # tt = (s + 36) * mask
                mb = msk_all[:, joff[t]:joff[t] + J] \
                    .rearrange("p (j a) -> p j a", a=1).to_broadcast([P, J, NH])
                tt = wp.tile([P, K * NH], F32, name="tt")
                nc.vector.scalar_tensor_tensor(
                    out=tt[:, 0:J * NH].rearrange("p (j h) -> p j h", j=J),
                    in0=s[:, 0:J * NH].rearrange("p (j h) -> p j h", j=J),
                    scalar=36.0, in1=mb, op0=ALU.add, op1=ALU.mult)

                # e = exp(0.25*tt - 9): compact f32 (for z) + expanded f16
                e_c = wp.tile([P, K * NH], F32, name="e_c")
                nc.scalar.activation(out=e_c[:, 0:J * NH], in_=tt[:, 0:J * NH],
                                     func=AF.Exp, bias=negq[:], scale=0.25)
                e_exp = wp.tile([P, K * HID], F16, name="e_exp")
                nc.scalar.activation(
                    out=e_exp[:, 0:J * HID].rearrange(
                        "p (j h d) -> p j h d", j=J, h=NH),
                    in_=tt[:, 0:J * NH].rearrange("p (j h) -> p j h", j=J)
                        .rearrange("p j (h a) -> p j h a", a=1)
                        .to_broadcast([P, J, NH, HD]),
                    func=AF.Exp, bias=negq[:], scale=0.25)

                # z[h] = sum_j e[j,h]
                z = wp.tile([P, NH], F32, name="z")
                nc.vector.tensor_reduce(
                    out=z[:],
                    in_=e_c[:, 0:J * NH].rearrange("p (j h) -> p h j", j=J),
                    axis=AX.X, op=ALU.add)
                zr = wp.tile([P, NH], F32, name="zr")
                nc.vector.reciprocal(out=zr[:], in_=z[:])

                prod2 = wp.tile([P, K * HID], F16, name="prod2")
                nc.vector.tensor_tensor(
                    out=prod2[:, 0:J * HID].rearrange("p (j c) -> p j c", j=J),
                    in0=vn,
                    in1=e_exp[:, 0:J * HID].rearrange("p (j c) -> p j c", j=J),
                    op=ALU.mult)

                # out tree over neighbors j (generic halving with odd carry)
                o = wp.tile([P, HID], F32, name="o")
                if J == 1:
                    nc.vector.tensor_copy(out=o[:], in_=prod2[:, 0:HID])
                else:
                    cur, curJ, lvl = prod2, J, 0
                    while curJ > 1:
                        half = curJ // 2
                        odd = curJ - 2 * half
                        nsz = half + odd
                        if nsz == 1:
                            nc.vector.tensor_tensor(
                                out=o[:], in0=cur[:, 0:HID],
                                in1=cur[:, HID:2 * HID], op=ALU.add)
                        else:
                            nt = wp.tile([P, 8 * HID], F16, name=f"oT{lvl}")
                            nc.vector.tensor_tensor(
                                out=nt[:, 0:half * HID],
                                in0=cur[:, 0:half * HID],
                                in1=cur[:, half * HID:2 * half * HID],
                                op=ALU.add)
                            if odd:
                                nc.vector.tensor_copy(
                                    out=nt[:, half * HID:nsz * HID],
                                    in_=cur[:, 2 * half * HID:curJ * HID])
                            cur = nt
                        curJ = nsz
                        lvl += 1

                ot = wp.tile([P, HID], F32, name="ot")
                nc.vector.tensor_tensor(
                    out=ot[:].rearrange("p (h d) -> p h d", h=NH),
                    in0=o[:].rearrange("p (h d) -> p h d", h=NH),
                    in1=zr[:].rearrange("p (h a) -> p h a", a=1)
                        .to_broadcast([P, NH, HD]),
                    op=ALU.mult)
                nc.sync.dma_start(out=out[r0:r0 + P, :], in_=ot[:])
    lp.__exit__(None, None, None)
    return nc


# ---------------------------------------------------------------- host side


def _prep(X, nbr_idx, nbr_mask, atom_emb, Wq, bq, Wk, bk, Wv, bv):
    X = np.asarray(X).astype(np.int64)
    N = X.shape[0]
    emb32 = np.asarray(atom_emb, dtype=np.float32)
    Wq = np.asarray(Wq, np.float32)
    Wk = np.asarray(Wk, np.float32)
    Wv = np.asarray(Wv, np.float32)
    bq = np.asarray(bq, np.float32).reshape(-1)
    bk = np.asarray(bk, np.float32).reshape(-1)
    bv = np.asarray(bv, np.float32).reshape(-1)

    # fused per-feature q|k|v tables; pair-packed; biases folded into table 8
    eq = emb32 @ Wq          # [9, VOCAB, HID]
    ek = emb32 @ Wk
    ev = emb32 @ Wv
    fused = np.concatenate([eq, ek, ev], axis=2)     # [9, VOCAB, 3H]
    tabs = []
    for f in range(4):
        pair = fused[2 * f][:, None, :] + fused[2 * f + 1][None, :, :]
        tabs.append(pair.reshape(NPAIR, QKV))
    t8 = fused[8] + np.concatenate([bq, bk, bv])[None, :]
    tabs.append(t8)
    emb_packed = np.ascontiguousarray(
        np.concatenate(tabs, axis=0).astype(np.float16))
    assert emb_packed.shape[0] == EMB_ROWS

    bases = np.array([0, NPAIR, 2 * NPAIR, 3 * NPAIR, 4 * NPAIR])
    xt = np.empty((N, 5), np.int32)
    for f in range(4):
        xt[:, f] = bases[f] + X[:, 2 * f] * VOCAB + X[:, 2 * f + 1]
    xt[:, 4] = bases[4] + X[:, 8]

    g = np.asarray(nbr_idx).astype(np.int64)
    mask = np.asarray(nbr_mask).astype(bool)
    u = mask.sum(axis=1)                             # unmasked counts

    # per-core node order: sort by unmasked count (stable)
    perms, inv_perms = [], []
    for r in range(NCORES):
        lo = r * NRC
        p = np.argsort(u[lo:lo + NRC], kind="stable")
        perms.append(p + lo)                         # global ids, sorted
        ip = np.empty(NRC, np.int64)
        ip[p] = np.arange(NRC)
        inv_perms.append(ip)

    # kv_full row of global node n (after per-core permutation)
    row_of = np.empty(N, np.int64)
    for r in range(NCORES):
        row_of[perms[r]] = r * NPC + np.arange(NRC)

    # per-core per-tile J (max unmasked in tile), then cross-core max
    j_tiles = np.zeros((NCORES, T), np.int64)
    for r in range(NCORES):
        uu = np.zeros(NPC, np.int64)
        uu[:NRC] = u[perms[r]]
        j_tiles[r] = uu.reshape(T, P).max(axis=1)
    j_list = np.maximum(j_tiles.max(axis=0), 1).astype(int)
    joff = np.concatenate([[0], np.cumsum(j_list)]).astype(int)
    sumj = int(j_list.sum())

    maps = []
    for r in range(NCORES):
        ids = perms[r]                               # sorted global node ids
        xcp = np.zeros((NPC, 5), np.int32)
        xcp[:NRC] = xt[ids]
        # pack unmasked neighbors first, per node; dummies are row 0 / mask 0
        nbp = np.zeros((P, sumj), np.int32)
        mkp = np.zeros((P, sumj), np.float16)
        rows_r = row_of[g[ids]]                      # [NRC, K] kv_full rows
        msk_r = mask[ids]                            # [NRC, K]
        for t in range(T):
            J = int(j_list[t])
            base = int(joff[t])
            for pp in range(P):
                i = t * P + pp
                if i >= NRC:
                    continue
                sel = rows_r[i][msk_r[i]]
                nslot = min(len(sel), J)
                nbp[pp, base:base + nslot] = sel[:nslot]
                mkp[pp, base:base + nslot] = 1.0
        maps.append({
            "xc": np.ascontiguousarray(
                xcp.reshape(T, P, 5).transpose(1, 0, 2).reshape(P, T * 5)),
            "nb": nbp, "mk": mkp, "emb": emb_packed,
        })

    # exact host patch for fully-masked rows: uniform average of v over the
    # ORIGINAL neighbor list (matches jax softmax of all -1e9)
    patches = []
    zrows = np.nonzero(u == 0)[0]
    if len(zrows):
        for n in zrows:
            nbrs = g[n]
            h_n = emb32[np.arange(N_FEATS)[None, :], X[nbrs]].sum(1)
            v_n = h_n @ Wv + bv[None, :]
            patches.append((int(n), v_n.mean(axis=0)))

    meta = {"j_list": [int(j) for j in j_list], "sumj": sumj, "perms": perms,
            "inv_perms": inv_perms, "patches": patches}
    return maps, meta


_CACHE = {}


def run_on_device(maps, meta, trace=False):
    from concourse.bass_utils import run_bass_kernel_spmd
    key = (tuple(meta["j_list"]), meta["sumj"])
    if _CACHE.get("key") != key:
        nc = bass.Bass()
        build(nc, meta["j_list"], meta["sumj"])
        _CACHE["nc"] = nc
        _CACHE["key"] = key
    return run_bass_kernel_spmd(_CACHE["nc"], maps, list(range(NCORES)),
                                trace=trace)


def kernel(X, nbr_idx, nbr_mask, atom_emb, Wq, bq, Wk, bk, Wv, bv):
    maps, meta = _prep(X, nbr_idx, nbr_mask, atom_emb, Wq, bq, Wk, bk, Wv, bv)
    res = run_on_device(maps, meta)
    outs = []
    for r in range(NCORES):
        o = res.results[r]["out"][:NRC]
        outs.append(o[meta["inv_perms"][r]])         # undo per-core sort
    full = np.concatenate(outs, axis=0)
    for n, v in meta["patches"]:
        full[n] = v
    return full


# revision 13
# speedup vs baseline: 1.0554x; 1.0554x over previous
"""TRN2 Bass kernel for nn_GTLayer (ELL sparse attention, N=50000, K=16).

Sharding: nodes split contiguously across 8 NeuronCores (6250/core, padded
to 6272 = 49 x 128), with each core's nodes re-ordered by unmasked-neighbor
count (outputs un-permuted on the host). Per core, three phases:
  1. q|k|v via host-fused lookup tables: T_f[v] = emb_f[v] @ [Wq|Wk|Wv]
     (+biases), feature-PAIR-packed -> 5 indirect-DMA row fetches per
     128-node tile + an f16 add-tree. No matmuls on device.
  2. AllGather of kv rows across the 8 cores (index/mask loads hidden).
  3. per tile: only the J_t = max-unmasked-count neighbor rows are gathered
     (masked neighbors contribute exp(-9)~0 and are skipped entirely;
     short rows are padded with dummy slots masked to 0). Attention runs
     in f16 on DVE with binary add-trees; exp on the Act engine.
Masking: tt=(s+36)*mask, e=exp(0.25*tt-9); dummy lanes get exp(-9)~1.2e-4.
Fully-masked rows are patched exactly on the host (mean of v over the
original neighbor list).
"""
import numpy as np

import concourse.bass as bass
import concourse.mybir as mybir
import concourse.tile as tile
from concourse.vector_clock import ScopedClock

F32 = mybir.dt.float32
I32 = mybir.dt.int32
F16 = mybir.dt.float16
AX = mybir.AxisListType
ALU = mybir.AluOpType
AF = mybir.ActivationFunctionType

N_FEATS, VOCAB, HID, NH, HD, K = 9, 119, 128, 8, 16, 16
P = 128
NCORES = 8
NRC = 6250          # real nodes per core
NPC = 6272          # padded nodes per core (49 x 128)
T = NPC // P
NPAIR = VOCAB * VOCAB
# fused q|k|v lookup table: 4 pair tables + 1 single table, 384-wide f16 rows
EMB_ROWS = 4 * NPAIR + VOCAB
QKV = 3 * HID

# ---------------------------------------------------------------- walrus fixes
# This walrus build rejects >1 sync-wait command per instruction. Two fixes:
# (1) TileContext tail drain: emit waits as single-wait nops.
# (2) General: split multi-wait instructions in the serialized BIR JSON by
#     inserting single-wait NoOps immediately before them (order preserved).


def _patched_drain_and_barrier(self, tick_clock, wait_clock):
    nc = self.nc
    probe = nc.sync.nop(nofuse=True)
    wait_clock.add_sem_waits(probe.ins, ScopedClock({None: tick_clock.global_clock}))
    waits = list(probe.ins.sync_info.on_wait or []) if probe.ins.sync_info else []
    if probe.ins.sync_info:
        probe.ins.sync_info.on_wait = waits[:1]
    for w in waits[1:]:
        n2 = nc.sync.nop(nofuse=True)
        if n2.ins.sync_info is None:
            n2.ins.sync_info = mybir.SyncInfo(on_update=[], on_wait=[w])
        else:
            n2.ins.sync_info.on_wait = [w]
    nc.sync.drain()
    nc.all_engine_barrier()
    assert self.sems is not None
    popped = nc._tile_sem_poison_stack.pop()
    assert popped is self._sem_poison
    nc.clear_and_free_semaphores(list(self.sems.allocated().values()))
    nc.all_engine_barrier()


tile.TileContext._drain_and_barrier = _patched_drain_and_barrier


def _split_waits_json(bir_bytes):
    import orjson
    m = orjson.loads(bir_bytes)
    n = 0
    for fn in m["functions"]:
        for blk in fn["blocks"]:
            new = []
            for ins in blk["instructions"]:
                si = ins.get("sync_info")
                waits = (si or {}).get("on_wait") or []
                if len(waits) > 1:
                    for w in waits[:-1]:
                        n += 1
                        new.append({
                            "debug": ins.get("debug", 0),
                            "engine": ins["engine"],
                            "ins": [], "name": f"I-wfix-{n}",
                            "opcode": "NoOp", "outs": [],
                            "sync_info": {"on_update": [], "on_wait": [w]},
                        })
                    si["on_wait"] = waits[-1:]
                new.append(ins)
            blk["instructions"] = new
    return orjson.dumps(m), n


import concourse.bass2jax as _b2j

_orig_cbk = _b2j.compile_bir_kernel


def _patched_cbk(ant_bir_str, *a, **kw):
    fixed, n = _split_waits_json(ant_bir_str)
    return _orig_cbk(fixed, *a, **kw)


_b2j.compile_bir_kernel = _patched_cbk

# ---------------------------------------------------------------- device code


def build(nc, j_list, sumj):
    ntot = NPC * NCORES

    xc = nc.dram_tensor("xc", [P, T * 4], I32, kind="ExternalInput")
    x8t = nc.dram_tensor("x8t", [1, T * P], F16, kind="ExternalInput")
    iota = nc.dram_tensor("iota", [VOCAB, 1], F16, kind="ExternalInput")
    nb = nc.dram_tensor("nb", [P, sumj], I32, kind="ExternalInput")
    mk = nc.dram_tensor("mk", [P, sumj], F16, kind="ExternalInput")
    emb = nc.dram_tensor("emb", [EMB_ROWS, QKV], F16, kind="ExternalInput")
    out = nc.dram_tensor("out", [NPC, HID], F32, kind="ExternalOutput")

    joff = np.concatenate([[0], np.cumsum(j_list)]).astype(int)

    lp = nc.allow_low_precision(reason="f16 attention pipeline")
    lp.__enter__()
    with tile.TileContext(nc) as tc:
        with (
            tc.tile_pool(name="const", bufs=1) as cp,
            tc.tile_pool(name="resident", bufs=1) as rp,
            tc.tile_pool(name="work", bufs=3) as wp,
            tc.tile_pool(name="gath", bufs=3) as gp,
            tc.tile_pool(name="psum", bufs=2, space="PSUM") as pp,
            tc.tile_pool(name="dram", bufs=1, space="DRAM") as dp,
        ):
            negq = cp.tile([P, 1], F32, name="negq")
            nc.gpsimd.memset(negq[:], -9.0)
            ones1 = cp.tile([1, VOCAB], F16, name="ones1")
            nc.vector.memset(ones1[:], 1.0)
            iota_c = cp.tile([VOCAB, 1], F16, name="iota_c")
            nc.sync.dma_start(out=iota_c[:], in_=iota[:])
            t8sb = cp.tile([VOCAB, QKV], F16, name="t8sb")
            nc.sync.dma_start(out=t8sb[:], in_=emb[4 * NPAIR:4 * NPAIR + VOCAB, :])

            xt_all = rp.tile([P, T * 4], I32, name="xt_all")
            nc.sync.dma_start(out=xt_all[:], in_=xc[:])
            x8_all = rp.tile([1, T * P], F16, name="x8_all")
            nc.sync.dma_start(out=x8_all[:], in_=x8t[:])
            idx_all = rp.tile([P, sumj], I32, name="idx_all")
            msk_all = rp.tile([P, sumj], F16, name="msk_all")
            qkv_all = rp.tile([P, T * QKV], F16, name="qkv_all")

            kv_shard = dp.tile([NPC, 2 * HID], F16, name="kv_shard")
            kv_full = dp.tile([ntot, 2 * HID], F16, name="kv_full",
                              addr_space="Shared")

            # ---------------- phase 1: fused q|k|v lookup + add-tree
            for t in range(T):
                r0 = t * P
                et = gp.tile([P, 4 * QKV], F16, name="et")
                for c in range(4):
                    nc.gpsimd.indirect_dma_start(
                        out=et[:, c * QKV:(c + 1) * QKV], out_offset=None,
                        in_=emb[:],
                        in_offset=bass.IndirectOffsetOnAxis(
                            ap=xt_all[:, t * 4 + c:t * 4 + c + 1], axis=0))
                # f8 chunk via one-hot matmul against the resident 119-row tab
                x8r_p = pp.tile([VOCAB, P], F32, name="x8r_p", space="PSUM")
                nc.tensor.matmul(out=x8r_p[:], lhsT=ones1[:],
                                 rhs=x8_all[:, t * P:(t + 1) * P],
                                 start=True, stop=True)
                oh8 = wp.tile([VOCAB, P], F16, name="oh8")
                nc.vector.tensor_tensor(
                    out=oh8[:], in0=x8r_p[:],
                    in1=iota_c[:].to_broadcast([VOCAB, P]), op=ALU.is_equal)
                q8_p = pp.tile([P, QKV], F32, name="q8_p", space="PSUM")
                nc.tensor.matmul(out=q8_p[:], lhsT=oh8[:], rhs=t8sb[:],
                                 start=True, stop=True)
                eL1 = wp.tile([P, 2 * QKV], F16, name="eL1")
                nc.vector.tensor_tensor(
                    out=eL1[:], in0=et[:, 0:2 * QKV],
                    in1=et[:, 2 * QKV:4 * QKV], op=ALU.add)
                eL2 = wp.tile([P, QKV], F16, name="eL2")
                nc.vector.tensor_tensor(
                    out=eL2[:], in0=eL1[:, 0:QKV], in1=eL1[:, QKV:2 * QKV],
                    op=ALU.add)
                nc.vector.tensor_tensor(
                    out=qkv_all[:, t * QKV:(t + 1) * QKV], in0=eL2[:],
                    in1=q8_p[:], op=ALU.add)
                nc.sync.dma_start(
                    out=kv_shard[r0:r0 + P, :],
                    in_=qkv_all[:, t * QKV + HID:(t + 1) * QKV])

            # ---------------- phase 2: allgather kv; loads hidden under it
            nc.gpsimd.collective_compute(
                "AllGather", ALU.bypass,
                replica_groups=[list(range(NCORES))],
                ins=[kv_shard[:]], outs=[kv_full[:]])

            nc.sync.dma_start(out=idx_all[:], in_=nb[:])
            nc.sync.dma_start(out=msk_all[:], in_=mk[:])

            # ---------------- phase 3: J_t neighbor gathers + attention
            def gather(t):
                J = j_list[t]
                knvn = gp.tile([P, K * 2 * HID], F16, name="knvn")
                for j in range(J):
                    o = joff[t] + j
                    nc.gpsimd.indirect_dma_start(
                        out=knvn[:, j * 2 * HID:(j + 1) * 2 * HID],
                        out_offset=None, in_=kv_full[:],
                        in_offset=bass.IndirectOffsetOnAxis(
                            ap=idx_all[:, o:o + 1], axis=0))
                return knvn

            knvn_next = gather(0)
            for t in range(T):
                r0 = t * P
                J = j_list[t]
                knvn = knvn_next
                if t + 1 < T:
                    knvn_next = gather(t + 1)
                kview = knvn[:].rearrange("p (j c) -> p j c", j=K)
                kn = kview[:, 0:J, 0:HID]
                vn = kview[:, 0:J, HID:2 * HID]

                qb = qkv_all[:, t * QKV:t * QKV + HID] \
                    .rearrange("p (a c) -> p a c", a=1).to_broadcast([P, J, HID])
                prod = wp.tile([P, K * HID], F16, name="prod")
                nc.vector.tensor_tensor(
                    out=prod[:, 0:J * HID].rearrange("p (j c) -> p j c", j=J),
                    in0=kn, in1=qb, op=ALU.mult)

                # score tree over head_dim d: 16 -> 8 -> 4 -> 2 -> 1
                pv = prod[:, 0:J * HID].rearrange(
                    "p (j h d) -> p j h d", j=J, h=NH)
                sL1 = wp.tile([P, K * NH * 8], F16, name="sL1")
                nc.vector.tensor_tensor(
                    out=sL1[:, 0:J * NH * 8].rearrange(
                        "p (j h d) -> p j h d", j=J, h=NH),
                    in0=pv[:, :, :, 0:8], in1=pv[:, :, :, 8:16], op=ALU.add)
                s1v = sL1[:, 0:J * NH * 8].rearrange(
                    "p (j h d) -> p j h d", j=J, h=NH)
                sL2 = wp.tile([P, K * NH * 4], F16, name="sL2")
                nc.vector.tensor_tensor(
                    out=sL2[:, 0:J * NH * 4].rearrange(
                        "p (j h d) -> p j h d", j=J, h=NH),
                    in0=s1v[:, :, :, 0:4], in1=s1v[:, :, :, 4:8], op=ALU.add)
                s2v = sL2[:, 0:J * NH * 4].rearrange(
                    "p (j h d) -> p j h d", j=J, h=NH)
                sL3 = wp.tile([P, K * NH * 2], F16, name="sL3")
                nc.vector.tensor_tensor(
                    out=sL3[:, 0:J * NH * 2].rearrange(
                        "p (j h d) -> p j h d", j=J, h=NH),
                    in0=s2v[:, :, :, 0:2], in1=s2v[:, :, :, 2:4], op=ALU.add)
                s3v = sL3[:, 0:J * NH * 2].rearrange(
                    "p (j h d) -> p j h d", j=J, h=NH)
                s = wp.tile([P, K * NH], F32, name="s")
                nc.vector.tensor_tensor(
                    out=s[:, 0:J * NH].rearrange(
                        "p (j h a) -> p j h a", j=J, h=NH),
                    in0=s3v[:, :, :, 0:1], in1=s3v[:, :, :, 1:2], op=ALU.add)

                # tt = (s + 36) * mask
                mb = msk_all[:, joff[t]:joff[t] + J] \
                    .rearrange("p (j a) -> p j a", a=1).to_broadcast([P, J, NH])
                tt = wp.tile([P, K * NH], F32, name="tt")
                nc.vector.scalar_tensor_tensor(
                    out=tt[:, 0:J * NH].rearrange("p (j h) -> p j h", j=J),
                    in0=s[:, 0:J * NH].rearrange("p (j h) -> p j h", j=J),
                    scalar=36.0, in1=mb, op0=ALU.add, op1=ALU.mult)

                # e = exp(0.25*tt - 9): compact f32 (for z) + expanded f16
                e_c = wp.tile([P, K * NH], F32, name="e_c")
                nc.scalar.activation(out=e_c[:, 0:J * NH], in_=tt[:, 0:J * NH],
                                     func=AF.Exp, bias=negq[:], scale=0.25)
                e_exp = wp.tile([P, K * HID], F16, name="e_exp")
                nc.scalar.activation(
                    out=e_exp[:, 0:J * HID].rearrange(
                        "p (j h d) -> p j h d", j=J, h=NH),
                    in_=tt[:, 0:J * NH].rearrange("p (j h) -> p j h", j=J)
                        .rearrange("p j (h a) -> p j h a", a=1)
                        .to_broadcast([P, J, NH, HD]),
                    func=AF.Exp, bias=negq[:], scale=0.25)

                # z[h] = sum_j e[j,h]
                z = wp.tile([P, NH], F32, name="z")
                nc.vector.tensor_reduce(
                    out=z[:],
                    in_=e_c[:, 0:J * NH].rearrange("p (j h) -> p h j", j=J),
                    axis=AX.X, op=ALU.add)
                zr = wp.tile([P, NH], F32, name="zr")
                nc.vector.reciprocal(out=zr[:], in_=z[:])

                prod2 = wp.tile([P, K * HID], F16, name="prod2")
                nc.vector.tensor_tensor(
                    out=prod2[:, 0:J * HID].rearrange("p (j c) -> p j c", j=J),
                    in0=vn,
                    in1=e_exp[:, 0:J * HID].rearrange("p (j c) -> p j c", j=J),
                    op=ALU.mult)

                # out tree over neighbors j (generic halving with odd carry)
                o = wp.tile([P, HID], F32, name="o")
                if J == 1:
                    nc.vector.tensor_copy(out=o[:], in_=prod2[:, 0:HID])
                else:
                    cur, curJ, lvl = prod2, J, 0
                    while curJ > 1:
                        half = curJ // 2
                        odd = curJ - 2 * half
                        nsz = half + odd
                        if nsz == 1:
                            nc.vector.tensor_tensor(
                                out=o[:], in0=cur[:, 0:HID],
                                in1=cur[:, HID:2 * HID], op=ALU.add)
                        else:
                            nt = wp.tile([P, 8 * HID], F16, name=f"oT{lvl}")
                            nc.vector.tensor_tensor(
                                out=nt[:, 0:half * HID],
                                in0=cur[:, 0:half * HID],
                                in1=cur[:, half * HID:2 * half * HID],
                                op=ALU.add)
                            if odd:
                                nc.vector.tensor_copy(
                                    out=nt[:, half * HID:nsz * HID],
                                    in_=cur[:, 2 * half * HID:curJ * HID])
                            cur = nt
                        curJ = nsz
                        lvl += 1

                ot = wp.tile([P, HID], F32, name="ot")
                nc.vector.tensor_tensor(
                    out=ot[:].rearrange("p (h d) -> p h d", h=NH),
                    in0=o[:].rearrange("p (h d) -> p h d", h=NH),
                    in1=zr[:].rearrange("p (h a) -> p h a", a=1)
                        .to_broadcast([P, NH, HD]),
                    op=ALU.mult)
                nc.sync.dma_start(out=out[r0:r0 + P, :], in_=ot[:])
    lp.__exit__(None, None, None)
    return nc


# ---------------------------------------------------------------- host side


def _prep(X, nbr_idx, nbr_mask, atom_emb, Wq, bq, Wk, bk, Wv, bv):
    X = np.asarray(X).astype(np.int64)
    N = X.shape[0]
    emb32 = np.asarray(atom_emb, dtype=np.float32)
    Wq = np.asarray(Wq, np.float32)
    Wk = np.asarray(Wk, np.float32)
    Wv = np.asarray(Wv, np.float32)
    bq = np.asarray(bq, np.float32).reshape(-1)
    bk = np.asarray(bk, np.float32).reshape(-1)
    bv = np.asarray(bv, np.float32).reshape(-1)

    # fused per-feature q|k|v tables; pair-packed; biases folded into table 8
    eq = emb32 @ Wq          # [9, VOCAB, HID]
    ek = emb32 @ Wk
    ev = emb32 @ Wv
    fused = np.concatenate([eq, ek, ev], axis=2)     # [9, VOCAB, 3H]
    tabs = []
    for f in range(4):
        pair = fused[2 * f][:, None, :] + fused[2 * f + 1][None, :, :]
        tabs.append(pair.reshape(NPAIR, QKV))
    t8 = fused[8] + np.concatenate([bq, bk, bv])[None, :]
    tabs.append(t8)
    emb_packed = np.ascontiguousarray(
        np.concatenate(tabs, axis=0).astype(np.float16))
    assert emb_packed.shape[0] == EMB_ROWS

    bases = np.array([0, NPAIR, 2 * NPAIR, 3 * NPAIR])
    xt = np.empty((N, 4), np.int32)
    for f in range(4):
        xt[:, f] = bases[f] + X[:, 2 * f] * VOCAB + X[:, 2 * f + 1]
    x8raw = X[:, 8].astype(np.float16)

    g = np.asarray(nbr_idx).astype(np.int64)
    mask = np.asarray(nbr_mask).astype(bool)
    u = mask.sum(axis=1)                             # unmasked counts

    # per-core node order: sort by unmasked count (stable)
    perms, inv_perms = [], []
    for r in range(NCORES):
        lo = r * NRC
        p = np.argsort(u[lo:lo + NRC], kind="stable")
        perms.append(p + lo)                         # global ids, sorted
        ip = np.empty(NRC, np.int64)
        ip[p] = np.arange(NRC)
        inv_perms.append(ip)

    # kv_full row of global node n (after per-core permutation)
    row_of = np.empty(N, np.int64)
    for r in range(NCORES):
        row_of[perms[r]] = r * NPC + np.arange(NRC)

    # per-core per-tile J (max unmasked in tile), then cross-core max
    j_tiles = np.zeros((NCORES, T), np.int64)
    for r in range(NCORES):
        uu = np.zeros(NPC, np.int64)
        uu[:NRC] = u[perms[r]]
        j_tiles[r] = uu.reshape(T, P).max(axis=1)
    j_list = np.maximum(j_tiles.max(axis=0), 1).astype(int)
    joff = np.concatenate([[0], np.cumsum(j_list)]).astype(int)
    sumj = int(j_list.sum())

    maps = []
    for r in range(NCORES):
        ids = perms[r]                               # sorted global node ids
        xcp = np.zeros((NPC, 4), np.int32)
        xcp[:NRC] = xt[ids]
        x8p = np.zeros(NPC, np.float16)
        x8p[:NRC] = x8raw[ids]
        # pack unmasked neighbors first, per node; dummies are row 0 / mask 0
        nbp = np.zeros((P, sumj), np.int32)
        mkp = np.zeros((P, sumj), np.float16)
        rows_r = row_of[g[ids]]                      # [NRC, K] kv_full rows
        msk_r = mask[ids]                            # [NRC, K]
        for t in range(T):
            J = int(j_list[t])
            base = int(joff[t])
            for pp in range(P):
                i = t * P + pp
                if i >= NRC:
                    continue
                sel = rows_r[i][msk_r[i]]
                nslot = min(len(sel), J)
                nbp[pp, base:base + nslot] = sel[:nslot]
                mkp[pp, base:base + nslot] = 1.0
        maps.append({
            "xc": np.ascontiguousarray(
                xcp.reshape(T, P, 4).transpose(1, 0, 2).reshape(P, T * 4)),
            "x8t": np.ascontiguousarray(x8p.reshape(1, T * P)),
            "iota": np.arange(VOCAB, dtype=np.float16).reshape(VOCAB, 1),
            "nb": nbp, "mk": mkp, "emb": emb_packed,
        })

    # exact host patch for fully-masked rows: uniform average of v over the
    # ORIGINAL neighbor list (matches jax softmax of all -1e9)
    patches = []
    zrows = np.nonzero(u == 0)[0]
    if len(zrows):
        for n in zrows:
            nbrs = g[n]
            h_n = emb32[np.arange(N_FEATS)[None, :], X[nbrs]].sum(1)
            v_n = h_n @ Wv + bv[None, :]
            patches.append((int(n), v_n.mean(axis=0)))

    meta = {"j_list": [int(j) for j in j_list], "sumj": sumj, "perms": perms,
            "inv_perms": inv_perms, "patches": patches}
    return maps, meta


_CACHE = {}


def run_on_device(maps, meta, trace=False):
    from concourse.bass_utils import run_bass_kernel_spmd
    key = (tuple(meta["j_list"]), meta["sumj"])
    if _CACHE.get("key") != key:
        nc = bass.Bass()
        build(nc, meta["j_list"], meta["sumj"])
        _CACHE["nc"] = nc
        _CACHE["key"] = key
    return run_bass_kernel_spmd(_CACHE["nc"], maps, list(range(NCORES)),
                                trace=trace)


def kernel(X, nbr_idx, nbr_mask, atom_emb, Wq, bq, Wk, bk, Wv, bv):
    maps, meta = _prep(X, nbr_idx, nbr_mask, atom_emb, Wq, bq, Wk, bk, Wv, bv)
    res = run_on_device(maps, meta)
    outs = []
    for r in range(NCORES):
        o = res.results[r]["out"][:NRC]
        outs.append(o[meta["inv_perms"][r]])         # undo per-core sort
    full = np.concatenate(outs, axis=0)
    for n, v in meta["patches"]:
        full[n] = v
    return full


# revision 14
# speedup vs baseline: 1.0696x; 1.0135x over previous
"""TRN2 Bass kernel for nn_GTLayer (ELL sparse attention, N=50000, K=16).

Sharding: nodes split contiguously across 8 NeuronCores (6250/core, padded
to 6272 = 49 x 128), with each core's nodes re-ordered by unmasked-neighbor
count (outputs un-permuted on the host). Per core, three phases:
  1. q|k|v via host-fused lookup tables: T_f[v] = emb_f[v] @ [Wq|Wk|Wv]
     (+biases), feature-PAIR-packed -> 5 indirect-DMA row fetches per
     128-node tile + an f16 add-tree. No matmuls on device.
  2. AllGather of kv rows across the 8 cores (index/mask loads hidden).
  3. per tile: only the J_t = max-unmasked-count neighbor rows are gathered
     (masked neighbors contribute exp(-9)~0 and are skipped entirely;
     short rows are padded with dummy slots masked to 0). Attention runs
     in f16 on DVE with binary add-trees; exp on the Act engine.
Masking: tt=(s+36)*mask, e=exp(0.25*tt-9); dummy lanes get exp(-9)~1.2e-4.
Fully-masked rows are patched exactly on the host (mean of v over the
original neighbor list).
"""
import numpy as np

import concourse.bass as bass
import concourse.mybir as mybir
import concourse.tile as tile
from concourse.vector_clock import ScopedClock

F32 = mybir.dt.float32
I32 = mybir.dt.int32
F16 = mybir.dt.float16
AX = mybir.AxisListType
ALU = mybir.AluOpType
AF = mybir.ActivationFunctionType

N_FEATS, VOCAB, HID, NH, HD, K = 9, 119, 128, 8, 16, 16
P = 128
NCORES = 8
NRC = 6250          # real nodes per core
NPC = 6272          # padded nodes per core (49 x 128)
T = NPC // P
NPAIR = VOCAB * VOCAB
# fused q|k|v lookup: 3 pair tables + 3 single tables, 384-wide f16 rows
EMB_ROWS = 3 * NPAIR + 3 * VOCAB
QKV = 3 * HID

# ---------------------------------------------------------------- walrus fixes
# This walrus build rejects >1 sync-wait command per instruction. Two fixes:
# (1) TileContext tail drain: emit waits as single-wait nops.
# (2) General: split multi-wait instructions in the serialized BIR JSON by
#     inserting single-wait NoOps immediately before them (order preserved).


def _patched_drain_and_barrier(self, tick_clock, wait_clock):
    nc = self.nc
    probe = nc.sync.nop(nofuse=True)
    wait_clock.add_sem_waits(probe.ins, ScopedClock({None: tick_clock.global_clock}))
    waits = list(probe.ins.sync_info.on_wait or []) if probe.ins.sync_info else []
    if probe.ins.sync_info:
        probe.ins.sync_info.on_wait = waits[:1]
    for w in waits[1:]:
        n2 = nc.sync.nop(nofuse=True)
        if n2.ins.sync_info is None:
            n2.ins.sync_info = mybir.SyncInfo(on_update=[], on_wait=[w])
        else:
            n2.ins.sync_info.on_wait = [w]
    nc.sync.drain()
    nc.all_engine_barrier()
    assert self.sems is not None
    popped = nc._tile_sem_poison_stack.pop()
    assert popped is self._sem_poison
    nc.clear_and_free_semaphores(list(self.sems.allocated().values()))
    nc.all_engine_barrier()


tile.TileContext._drain_and_barrier = _patched_drain_and_barrier


def _split_waits_json(bir_bytes):
    import orjson
    m = orjson.loads(bir_bytes)
    n = 0
    for fn in m["functions"]:
        for blk in fn["blocks"]:
            new = []
            for ins in blk["instructions"]:
                si = ins.get("sync_info")
                waits = (si or {}).get("on_wait") or []
                if len(waits) > 1:
                    for w in waits[:-1]:
                        n += 1
                        new.append({
                            "debug": ins.get("debug", 0),
                            "engine": ins["engine"],
                            "ins": [], "name": f"I-wfix-{n}",
                            "opcode": "NoOp", "outs": [],
                            "sync_info": {"on_update": [], "on_wait": [w]},
                        })
                    si["on_wait"] = waits[-1:]
                new.append(ins)
            blk["instructions"] = new
    return orjson.dumps(m), n


import concourse.bass2jax as _b2j

_orig_cbk = _b2j.compile_bir_kernel


def _patched_cbk(ant_bir_str, *a, **kw):
    fixed, n = _split_waits_json(ant_bir_str)
    return _orig_cbk(fixed, *a, **kw)


_b2j.compile_bir_kernel = _patched_cbk

# ---------------------------------------------------------------- device code


def build(nc, j_list, sumj):
    ntot = NPC * NCORES

    xc = nc.dram_tensor("xc", [P, T * 3], I32, kind="ExternalInput")
    x8t = nc.dram_tensor("x8t", [1, T * 3 * P], F16, kind="ExternalInput")
    iota = nc.dram_tensor("iota", [VOCAB, 1], F16, kind="ExternalInput")
    nb = nc.dram_tensor("nb", [P, sumj], I32, kind="ExternalInput")
    mk = nc.dram_tensor("mk", [P, sumj], F16, kind="ExternalInput")
    emb = nc.dram_tensor("emb", [EMB_ROWS, QKV], F16, kind="ExternalInput")
    out = nc.dram_tensor("out", [NPC, HID], F32, kind="ExternalOutput")

    joff = np.concatenate([[0], np.cumsum(j_list)]).astype(int)

    lp = nc.allow_low_precision(reason="f16 attention pipeline")
    lp.__enter__()
    with tile.TileContext(nc) as tc:
        with (
            tc.tile_pool(name="const", bufs=1) as cp,
            tc.tile_pool(name="resident", bufs=1) as rp,
            tc.tile_pool(name="work", bufs=3) as wp,
            tc.tile_pool(name="gath", bufs=3) as gp,
            tc.tile_pool(name="psum", bufs=2, space="PSUM") as pp,
            tc.tile_pool(name="dram", bufs=1, space="DRAM") as dp,
        ):
            negq = cp.tile([P, 1], F32, name="negq")
            nc.gpsimd.memset(negq[:], -9.0)
            ones1 = cp.tile([1, VOCAB], F16, name="ones1")
            nc.vector.memset(ones1[:], 1.0)
            iota_c = cp.tile([VOCAB, 1], F16, name="iota_c")
            nc.sync.dma_start(out=iota_c[:], in_=iota[:])
            tsb = []
            for i in range(3):
                tb = cp.tile([VOCAB, QKV], F16, name=f"t{i}sb")
                nc.sync.dma_start(
                    out=tb[:], in_=emb[3 * NPAIR + i * VOCAB:
                                       3 * NPAIR + (i + 1) * VOCAB, :])
                tsb.append(tb)

            xt_all = rp.tile([P, T * 3], I32, name="xt_all")
            nc.sync.dma_start(out=xt_all[:], in_=xc[:])
            x8_all = rp.tile([1, T * 3 * P], F16, name="x8_all")
            nc.sync.dma_start(out=x8_all[:], in_=x8t[:])
            idx_all = rp.tile([P, sumj], I32, name="idx_all")
            msk_all = rp.tile([P, sumj], F16, name="msk_all")
            qkv_all = rp.tile([P, T * QKV], F16, name="qkv_all")

            kv_shard = dp.tile([NPC, 2 * HID], F16, name="kv_shard")
            kv_full = dp.tile([ntot, 2 * HID], F16, name="kv_full",
                              addr_space="Shared")

            # ---------------- phase 1: fused q|k|v lookup + add-tree
            for t in range(T):
                r0 = t * P
                et = gp.tile([P, 3 * QKV], F16, name="et")
                for c in range(3):
                    nc.gpsimd.indirect_dma_start(
                        out=et[:, c * QKV:(c + 1) * QKV], out_offset=None,
                        in_=emb[:],
                        in_offset=bass.IndirectOffsetOnAxis(
                            ap=xt_all[:, t * 3 + c:t * 3 + c + 1], axis=0))
                # f6/f7/f8 chunks via one-hot matmuls on resident 119-row tabs
                x8r_p = pp.tile([VOCAB, 3 * P], F32, name="x8r_p", space="PSUM")
                nc.tensor.matmul(out=x8r_p[:], lhsT=ones1[:],
                                 rhs=x8_all[:, t * 3 * P:(t + 1) * 3 * P],
                                 start=True, stop=True)
                oh8 = wp.tile([VOCAB, 3 * P], F16, name="oh8")
                nc.vector.tensor_tensor(
                    out=oh8[:], in0=x8r_p[:],
                    in1=iota_c[:].to_broadcast([VOCAB, 3 * P]),
                    op=ALU.is_equal)
                q8_p = pp.tile([P, QKV], F32, name="q8_p", space="PSUM")
                for i in range(3):
                    nc.tensor.matmul(out=q8_p[:],
                                     lhsT=oh8[:, i * P:(i + 1) * P],
                                     rhs=tsb[i][:],
                                     start=(i == 0), stop=(i == 2))
                eL1 = wp.tile([P, QKV], F16, name="eL1")
                nc.vector.tensor_tensor(
                    out=eL1[:], in0=et[:, 0:QKV], in1=et[:, QKV:2 * QKV],
                    op=ALU.add)
                eL2 = wp.tile([P, QKV], F16, name="eL2")
                nc.vector.tensor_tensor(
                    out=eL2[:], in0=eL1[:], in1=et[:, 2 * QKV:3 * QKV],
                    op=ALU.add)
                nc.vector.tensor_tensor(
                    out=qkv_all[:, t * QKV:(t + 1) * QKV], in0=eL2[:],
                    in1=q8_p[:], op=ALU.add)
                nc.sync.dma_start(
                    out=kv_shard[r0:r0 + P, :],
                    in_=qkv_all[:, t * QKV + HID:(t + 1) * QKV])

            # ---------------- phase 2: allgather kv; loads hidden under it
            nc.gpsimd.collective_compute(
                "AllGather", ALU.bypass,
                replica_groups=[list(range(NCORES))],
                ins=[kv_shard[:]], outs=[kv_full[:]])

            nc.sync.dma_start(out=idx_all[:], in_=nb[:])
            nc.sync.dma_start(out=msk_all[:], in_=mk[:])

            # ---------------- phase 3: J_t neighbor gathers + attention
            def gather(t):
                J = j_list[t]
                knvn = gp.tile([P, K * 2 * HID], F16, name="knvn")
                for j in range(J):
                    o = joff[t] + j
                    nc.gpsimd.indirect_dma_start(
                        out=knvn[:, j * 2 * HID:(j + 1) * 2 * HID],
                        out_offset=None, in_=kv_full[:],
                        in_offset=bass.IndirectOffsetOnAxis(
                            ap=idx_all[:, o:o + 1], axis=0))
                return knvn

            knvn_next = gather(0)
            for t in range(T):
                r0 = t * P
                J = j_list[t]
                knvn = knvn_next
                if t + 1 < T:
                    knvn_next = gather(t + 1)
                kview = knvn[:].rearrange("p (j c) -> p j c", j=K)
                kn = kview[:, 0:J, 0:HID]
                vn = kview[:, 0:J, HID:2 * HID]

                qb = qkv_all[:, t * QKV:t * QKV + HID] \
                    .rearrange("p (a c) -> p a c", a=1).to_broadcast([P, J, HID])
                prod = wp.tile([P, K * HID], F16, name="prod")
                nc.vector.tensor_tensor(
                    out=prod[:, 0:J * HID].rearrange("p (j c) -> p j c", j=J),
                    in0=kn, in1=qb, op=ALU.mult)

                # score tree over head_dim d: 16 -> 8 -> 4 -> 2 -> 1
                pv = prod[:, 0:J * HID].rearrange(
                    "p (j h d) -> p j h d", j=J, h=NH)
                sL1 = wp.tile([P, K * NH * 8], F16, name="sL1")
                nc.vector.tensor_tensor(
                    out=sL1[:, 0:J * NH * 8].rearrange(
                        "p (j h d) -> p j h d", j=J, h=NH),
                    in0=pv[:, :, :, 0:8], in1=pv[:, :, :, 8:16], op=ALU.add)
                s1v = sL1[:, 0:J * NH * 8].rearrange(
                    "p (j h d) -> p j h d", j=J, h=NH)
                sL2 = wp.tile([P, K * NH * 4], F16, name="sL2")
                nc.vector.tensor_tensor(
                    out=sL2[:, 0:J * NH * 4].rearrange(
                        "p (j h d) -> p j h d", j=J, h=NH),
                    in0=s1v[:, :, :, 0:4], in1=s1v[:, :, :, 4:8], op=ALU.add)
                s2v = sL2[:, 0:J * NH * 4].rearrange(
                    "p (j h d) -> p j h d", j=J, h=NH)
                sL3 = wp.tile([P, K * NH * 2], F16, name="sL3")
                nc.vector.tensor_tensor(
                    out=sL3[:, 0:J * NH * 2].rearrange(
                        "p (j h d) -> p j h d", j=J, h=NH),
                    in0=s2v[:, :, :, 0:2], in1=s2v[:, :, :, 2:4], op=ALU.add)
                s3v = sL3[:, 0:J * NH * 2].rearrange(
                    "p (j h d) -> p j h d", j=J, h=NH)
                s = wp.tile([P, K * NH], F32, name="s")
                nc.vector.tensor_tensor(
                    out=s[:, 0:J * NH].rearrange(
                        "p (j h a) -> p j h a", j=J, h=NH),
                    in0=s3v[:, :, :, 0:1], in1=s3v[:, :, :, 1:2], op=ALU.add)

                # tt = (s + 36) * mask
                mb = msk_all[:, joff[t]:joff[t] + J] \
                    .rearrange("p (j a) -> p j a", a=1).to_broadcast([P, J, NH])
                tt = wp.tile([P, K * NH], F32, name="tt")
                nc.vector.scalar_tensor_tensor(
                    out=tt[:, 0:J * NH].rearrange("p (j h) -> p j h", j=J),
                    in0=s[:, 0:J * NH].rearrange("p (j h) -> p j h", j=J),
                    scalar=36.0, in1=mb, op0=ALU.add, op1=ALU.mult)

                # e = exp(0.25*tt - 9): compact f32 (for z) + expanded f16
                e_c = wp.tile([P, K * NH], F32, name="e_c")
                nc.scalar.activation(out=e_c[:, 0:J * NH], in_=tt[:, 0:J * NH],
                                     func=AF.Exp, bias=negq[:], scale=0.25)
                e_exp = wp.tile([P, K * HID], F16, name="e_exp")
                nc.scalar.activation(
                    out=e_exp[:, 0:J * HID].rearrange(
                        "p (j h d) -> p j h d", j=J, h=NH),
                    in_=tt[:, 0:J * NH].rearrange("p (j h) -> p j h", j=J)
                        .rearrange("p j (h a) -> p j h a", a=1)
                        .to_broadcast([P, J, NH, HD]),
                    func=AF.Exp, bias=negq[:], scale=0.25)

                # z[h] = sum_j e[j,h]
                z = wp.tile([P, NH], F32, name="z")
                nc.vector.tensor_reduce(
                    out=z[:],
                    in_=e_c[:, 0:J * NH].rearrange("p (j h) -> p h j", j=J),
                    axis=AX.X, op=ALU.add)
                zr = wp.tile([P, NH], F32, name="zr")
                nc.vector.reciprocal(out=zr[:], in_=z[:])

                prod2 = wp.tile([P, K * HID], F16, name="prod2")
                nc.vector.tensor_tensor(
                    out=prod2[:, 0:J * HID].rearrange("p (j c) -> p j c", j=J),
                    in0=vn,
                    in1=e_exp[:, 0:J * HID].rearrange("p (j c) -> p j c", j=J),
                    op=ALU.mult)

                # out tree over neighbors j (generic halving with odd carry)
                o = wp.tile([P, HID], F32, name="o")
                if J == 1:
                    nc.vector.tensor_copy(out=o[:], in_=prod2[:, 0:HID])
                else:
                    cur, curJ, lvl = prod2, J, 0
                    while curJ > 1:
                        half = curJ // 2
                        odd = curJ - 2 * half
                        nsz = half + odd
                        if nsz == 1:
                            nc.vector.tensor_tensor(
                                out=o[:], in0=cur[:, 0:HID],
                                in1=cur[:, HID:2 * HID], op=ALU.add)
                        else:
                            nt = wp.tile([P, 8 * HID], F16, name=f"oT{lvl}")
                            nc.vector.tensor_tensor(
                                out=nt[:, 0:half * HID],
                                in0=cur[:, 0:half * HID],
                                in1=cur[:, half * HID:2 * half * HID],
                                op=ALU.add)
                            if odd:
                                nc.vector.tensor_copy(
                                    out=nt[:, half * HID:nsz * HID],
                                    in_=cur[:, 2 * half * HID:curJ * HID])
                            cur = nt
                        curJ = nsz
                        lvl += 1

                ot = wp.tile([P, HID], F32, name="ot")
                nc.vector.tensor_tensor(
                    out=ot[:].rearrange("p (h d) -> p h d", h=NH),
                    in0=o[:].rearrange("p (h d) -> p h d", h=NH),
                    in1=zr[:].rearrange("p (h a) -> p h a", a=1)
                        .to_broadcast([P, NH, HD]),
                    op=ALU.mult)
                nc.sync.dma_start(out=out[r0:r0 + P, :], in_=ot[:])
    lp.__exit__(None, None, None)
    return nc


# ---------------------------------------------------------------- host side


def _prep(X, nbr_idx, nbr_mask, atom_emb, Wq, bq, Wk, bk, Wv, bv):
    X = np.asarray(X).astype(np.int64)
    N = X.shape[0]
    emb32 = np.asarray(atom_emb, dtype=np.float32)
    Wq = np.asarray(Wq, np.float32)
    Wk = np.asarray(Wk, np.float32)
    Wv = np.asarray(Wv, np.float32)
    bq = np.asarray(bq, np.float32).reshape(-1)
    bk = np.asarray(bk, np.float32).reshape(-1)
    bv = np.asarray(bv, np.float32).reshape(-1)

    # fused per-feature q|k|v tables; pair-packed; biases folded into table 8
    eq = emb32 @ Wq          # [9, VOCAB, HID]
    ek = emb32 @ Wk
    ev = emb32 @ Wv
    fused = np.concatenate([eq, ek, ev], axis=2)     # [9, VOCAB, 3H]
    tabs = []
    for f in range(3):
        pair = fused[2 * f][:, None, :] + fused[2 * f + 1][None, :, :]
        tabs.append(pair.reshape(NPAIR, QKV))
    tabs.append(fused[6])
    tabs.append(fused[7])
    tabs.append(fused[8] + np.concatenate([bq, bk, bv])[None, :])
    emb_packed = np.ascontiguousarray(
        np.concatenate(tabs, axis=0).astype(np.float16))
    assert emb_packed.shape[0] == EMB_ROWS

    bases = np.array([0, NPAIR, 2 * NPAIR])
    xt = np.empty((N, 3), np.int32)
    for f in range(3):
        xt[:, f] = bases[f] + X[:, 2 * f] * VOCAB + X[:, 2 * f + 1]
    x678raw = X[:, 6:9].astype(np.float16)           # [N, 3]

    g = np.asarray(nbr_idx).astype(np.int64)
    mask = np.asarray(nbr_mask).astype(bool)
    u = mask.sum(axis=1)                             # unmasked counts

    # per-core node order: sort by unmasked count (stable)
    perms, inv_perms = [], []
    for r in range(NCORES):
        lo = r * NRC
        p = np.argsort(u[lo:lo + NRC], kind="stable")
        perms.append(p + lo)                         # global ids, sorted
        ip = np.empty(NRC, np.int64)
        ip[p] = np.arange(NRC)
        inv_perms.append(ip)

    # kv_full row of global node n (after per-core permutation)
    row_of = np.empty(N, np.int64)
    for r in range(NCORES):
        row_of[perms[r]] = r * NPC + np.arange(NRC)

    # per-core per-tile J (max unmasked in tile), then cross-core max
    j_tiles = np.zeros((NCORES, T), np.int64)
    for r in range(NCORES):
        uu = np.zeros(NPC, np.int64)
        uu[:NRC] = u[perms[r]]
        j_tiles[r] = uu.reshape(T, P).max(axis=1)
    j_list = np.maximum(j_tiles.max(axis=0), 1).astype(int)
    joff = np.concatenate([[0], np.cumsum(j_list)]).astype(int)
    sumj = int(j_list.sum())

    maps = []
    for r in range(NCORES):
        ids = perms[r]                               # sorted global node ids
        xcp = np.zeros((NPC, 3), np.int32)
        xcp[:NRC] = xt[ids]
        x8p = np.zeros((NPC, 3), np.float16)
        x8p[:NRC] = x678raw[ids]
        # pack unmasked neighbors first, per node; dummies are row 0 / mask 0
        nbp = np.zeros((P, sumj), np.int32)
        mkp = np.zeros((P, sumj), np.float16)
        rows_r = row_of[g[ids]]                      # [NRC, K] kv_full rows
        msk_r = mask[ids]                            # [NRC, K]
        for t in range(T):
            J = int(j_list[t])
            base = int(joff[t])
            for pp in range(P):
                i = t * P + pp
                if i >= NRC:
                    continue
                sel = rows_r[i][msk_r[i]]
                nslot = min(len(sel), J)
                nbp[pp, base:base + nslot] = sel[:nslot]
                mkp[pp, base:base + nslot] = 1.0
        maps.append({
            "xc": np.ascontiguousarray(
                xcp.reshape(T, P, 3).transpose(1, 0, 2).reshape(P, T * 3)),
            "x8t": np.ascontiguousarray(
                x8p.reshape(T, P, 3).transpose(0, 2, 1).reshape(1, T * 3 * P)),
            "iota": np.arange(VOCAB, dtype=np.float16).reshape(VOCAB, 1),
            "nb": nbp, "mk": mkp, "emb": emb_packed,
        })

    # exact host patch for fully-masked rows: uniform average of v over the
    # ORIGINAL neighbor list (matches jax softmax of all -1e9)
    patches = []
    zrows = np.nonzero(u == 0)[0]
    if len(zrows):
        for n in zrows:
            nbrs = g[n]
            h_n = emb32[np.arange(N_FEATS)[None, :], X[nbrs]].sum(1)
            v_n = h_n @ Wv + bv[None, :]
            patches.append((int(n), v_n.mean(axis=0)))

    meta = {"j_list": [int(j) for j in j_list], "sumj": sumj, "perms": perms,
            "inv_perms": inv_perms, "patches": patches}
    return maps, meta


_CACHE = {}


def run_on_device(maps, meta, trace=False):
    from concourse.bass_utils import run_bass_kernel_spmd
    key = (tuple(meta["j_list"]), meta["sumj"])
    if _CACHE.get("key") != key:
        nc = bass.Bass()
        build(nc, meta["j_list"], meta["sumj"])
        _CACHE["nc"] = nc
        _CACHE["key"] = key
    return run_bass_kernel_spmd(_CACHE["nc"], maps, list(range(NCORES)),
                                trace=trace)


def kernel(X, nbr_idx, nbr_mask, atom_emb, Wq, bq, Wk, bk, Wv, bv):
    maps, meta = _prep(X, nbr_idx, nbr_mask, atom_emb, Wq, bq, Wk, bk, Wv, bv)
    res = run_on_device(maps, meta)
    outs = []
    for r in range(NCORES):
        o = res.results[r]["out"][:NRC]
        outs.append(o[meta["inv_perms"][r]])         # undo per-core sort
    full = np.concatenate(outs, axis=0)
    for n, v in meta["patches"]:
        full[n] = v
    return full
